# revision 9
# baseline (speedup 1.0000x reference)
"""Trainium2 Bass kernel for nn_GCNBert (GCN + protein-BERT co-attention).

Contract: kernel(**inputs) takes the FULL unsharded inputs (numpy) and
returns the FULL [128, 1] float32 output.  Internally the batch of 128
graphs is sharded across 8 NeuronCores (16 graphs each); the small weights
are replicated; the protein table lookup (a pure data-movement gather) is
done host-side so each core only holds the 16 protein rows it needs.

All FLOPs of the model run on-device:
  - 3 GCN layers (dense per-graph normalized adjacency, precomputed on host
    from edge indices only), fc1/fc2 over the dense node batch
  - protein branch bert1/bert2
  - parallel co-attention (C, H_c, H_p, softmaxes, weighted sums)
  - cat1/cat2/out head
"""

import os
import sys

import numpy as np

sys.path.insert(0, "/opt/trn_rl_repo")

B = 128
NPG = 40
N = B * NPG
LP = 512
MAXN = 45
N_CORES = 8
G = B // N_CORES          # graphs per core
NODES = G * MAXN          # dense node slots per core (720)
DBERT = 1280
KC_BERT = DBERT // 128    # 10

# numeric modes (validated empirically against the fp32 reference)
USE_F32R = os.environ.get("GCNBERT_F32R", "1") == "1"
PHASES = os.environ.get("GCNBERT_PHASES", "pgch")
CSTEP = int(os.environ.get("GCNBERT_CSTEP", "9"))
TT_BF16 = os.environ.get("GCNBERT_TT_BF16", "1") == "1"

_PROGRAM_CACHE = {}


def _build_program():
    import concourse.bass as bass
    import concourse.mybir as mybir
    import concourse.tile as tile
    from concourse import bacc

    dt = mybir.dt
    f32 = dt.float32
    tt_dt = dt.bfloat16 if TT_BF16 else f32
    ACT = mybir.ActivationFunctionType

    def mmt(ap):
        # bitcast fp32 matmul operands to float32r (full-rate PE mode)
        if USE_F32R and ap.dtype == f32:
            return ap.bitcast(dt.float32r)
        return ap

    nc = bacc.Bacc()

    # ---- DRAM parameters (per-core shapes) ----
    xt_d = nc.declare_dram_parameter("xt", [78, NODES], f32, isOutput=False)
    mt_d = nc.declare_dram_parameter("mt", [MAXN, NODES], f32, isOutput=False)
    tt_d = nc.declare_dram_parameter("tt", [G, 128, KC_BERT, LP], tt_dt, isOutput=False)

    gw1_d = nc.declare_dram_parameter("gw1", [78, 128], f32, isOutput=False)
    gw2_d = nc.declare_dram_parameter("gw2", [128, 128], f32, isOutput=False)
    gw3_d = nc.declare_dram_parameter("gw3", [128, 256], f32, isOutput=False)
    gb1_d = nc.declare_dram_parameter("gb1", [128, 1], f32, isOutput=False)
    gb2_d = nc.declare_dram_parameter("gb2", [128, 1], f32, isOutput=False)
    gb3_d = nc.declare_dram_parameter("gb3", [128, 2], f32, isOutput=False)
    fc1w_d = nc.declare_dram_parameter("fc1w", [128, 2, 1024], f32, isOutput=False)
    fc1b_d = nc.declare_dram_parameter("fc1b", [128, 8], f32, isOutput=False)
    fc2w_d = nc.declare_dram_parameter("fc2w", [128, 8, 128], f32, isOutput=False)
    fc2b_d = nc.declare_dram_parameter("fc2b", [128, 1], f32, isOutput=False)
    b1w_d = nc.declare_dram_parameter("b1w", [128, KC_BERT, 128], tt_dt, isOutput=False)
    b1b_d = nc.declare_dram_parameter("b1b", [128, 1], f32, isOutput=False)
    b2w_d = nc.declare_dram_parameter("b2w", [128, 128], f32, isOutput=False)
    b2b_d = nc.declare_dram_parameter("b2b", [128, 1], f32, isOutput=False)
    wb_d = nc.declare_dram_parameter("wb", [128, 128], f32, isOutput=False)
    wct_d = nc.declare_dram_parameter("wct", [128, 32], f32, isOutput=False)
    wpt_d = nc.declare_dram_parameter("wpt", [128, 32], f32, isOutput=False)
    whc_d = nc.declare_dram_parameter("whc", [32, 1], f32, isOutput=False)
    whp_d = nc.declare_dram_parameter("whp", [32, 1], f32, isOutput=False)
    c1w_d = nc.declare_dram_parameter("c1w", [128, 2, 1024], f32, isOutput=False)
    c1b_d = nc.declare_dram_parameter("c1b", [128, 8], f32, isOutput=False)
    c2w_d = nc.declare_dram_parameter("c2w", [128, 8, 512], f32, isOutput=False)
    c2b_d = nc.declare_dram_parameter("c2b", [128, 4], f32, isOutput=False)
    ow_d = nc.declare_dram_parameter("ow", [128, 4], f32, isOutput=False)
    ob_d = nc.declare_dram_parameter("ob", [1, 1], f32, isOutput=False)

    out_d = nc.declare_dram_parameter("out", [1, G], f32, isOutput=True)

    with tile.TileContext(nc) as tc:
        with (
            tc.tile_pool(name="singles", bufs=1) as singles,
            tc.tile_pool(name="longs", bufs=1) as longs,
            tc.tile_pool(name="work", bufs=3) as work,
            tc.tile_pool(name="tstream", bufs=2) as tstream,
        ):
            # ---- load weights / constants ----
            def load(pool, dram, shape, dtype=f32, tag=None):
                t = pool.tile(shape, dtype, tag=tag or dram.name + "_sb")
                nc.sync.dma_start(out=t, in_=dram[:])
                return t

            xt_sb = load(singles, xt_d, [78, NODES])
            mt_sb = load(singles, mt_d, [MAXN, NODES])
            gw1_sb = load(singles, gw1_d, [78, 128])
            gw2_sb = load(singles, gw2_d, [128, 128])
            gw3_sb = load(singles, gw3_d, [128, 256])
            gb1_sb = load(singles, gb1_d, [128, 1])
            gb2_sb = load(singles, gb2_d, [128, 1])
            gb3_sb = load(singles, gb3_d, [128, 2])
            fc1w_sb = load(singles, fc1w_d, [128, 2, 1024])
            fc1b_sb = load(singles, fc1b_d, [128, 8])
            fc2w_sb = load(singles, fc2w_d, [128, 8, 128])
            fc2b_sb = load(singles, fc2b_d, [128, 1])
            b1w_sb = load(singles, b1w_d, [128, KC_BERT, 128], tt_dt)
            b1b_sb = load(singles, b1b_d, [128, 1])
            b2w_sb = load(singles, b2w_d, [128, 128])
            b2b_sb = load(singles, b2b_d, [128, 1])
            wb_sb = load(singles, wb_d, [128, 128])
            wct_sb = load(singles, wct_d, [128, 32])
            wpt_sb = load(singles, wpt_d, [128, 32])
            whc_sb = load(singles, whc_d, [32, 1])
            whp_sb = load(singles, whp_d, [32, 1])
            c1w_sb = load(singles, c1w_d, [128, 2, 1024])
            c1b_sb = load(singles, c1b_d, [128, 8])
            c2w_sb = load(singles, c2w_d, [128, 8, 512])
            c2b_sb = load(singles, c2b_d, [128, 4])
            ow_sb = load(singles, ow_d, [128, 4])
            ob_sb = load(singles, ob_d, [1, 1])

            ones_sb = singles.tile([1, 128], f32, tag="ones")
            nc.vector.memset(ones_sb, 1.0)

            # ---- long-lived activations ----
            ttall_sb = longs.tile([128, G * LP], f32, tag="ttall")  # t.T per graph
            xt2_sb = longs.tile([128, NODES], f32, tag="xt2")
            xt3_sb = longs.tile([128, NODES], f32, tag="xt3")
            xt4_sb = longs.tile([128, 2, NODES], f32, tag="xt4")
            y1_sb = longs.tile([128, 8, NODES], f32, tag="y1")
            xc_sb = longs.tile([128, NODES], f32, tag="xc")
            cpc_sb = longs.tile([128, G], f32, tag="cpc")
            cpp_sb = longs.tile([128, G], f32, tag="cpp")
            y1h_sb = longs.tile([128, 8, G], f32, tag="y1h")
            y2h_sb = longs.tile([128, 4, G], f32, tag="y2h")
            outh_sb = longs.tile([1, G], f32, tag="outh")

            with tc.tile_pool(name="psumA", bufs=2, space="PSUM") as psA:
                # ============ protein branch ============
                for i in range(G if "p" in PHASES else 0):
                    tha = tstream.tile([128, 5, LP], tt_dt, tag="tch")
                    nc.sync.dma_start(out=tha, in_=tt_d[i, :, 0:5, :])
                    thb = tstream.tile([128, 5, LP], tt_dt, tag="tch")
                    nc.sync.dma_start(out=thb, in_=tt_d[i, :, 5:10, :])
                    p1 = psA.tile([128, LP], f32, tag="pt1")
                    for kc in range(KC_BERT):
                        src = tha if kc < 5 else thb
                        nc.tensor.matmul(
                            p1,
                            mmt(b1w_sb[:, kc, :]),
                            mmt(src[:, kc % 5, :]),
                            start=(kc == 0),
                            stop=(kc == KC_BERT - 1),
                        )
                    t1_sb = work.tile([128, LP], f32, tag="t1sb")
                    nc.scalar.activation(
                        out=t1_sb, in_=p1, func=ACT.Relu, bias=b1b_sb[:, 0:1]
                    )
                    p2 = psA.tile([128, LP], f32, tag="pt2")
                    nc.tensor.matmul(p2, mmt(b2w_sb), mmt(t1_sb), start=True, stop=True)
                    nc.scalar.activation(
                        out=ttall_sb[:, i * LP : (i + 1) * LP],
                        in_=p2,
                        func=ACT.Relu,
                        bias=b2b_sb[:, 0:1],
                    )

                # ============ GCN stack (feature-major chain) ============
                # layer l: for each graph g:
                #   xw_g   = matmul(lhsT=XT_l[:, g], rhs=gWl)      -> [45, Fout] (node-major)
                #   hT_g   = matmul(lhsT=xw_g, rhs=MT_g)           -> [Fout, 45] (feature-major)
                #   XT_l+1[:, g] = relu(hT_g + b)
                layer_cfg = [
                    (xt_sb, 78, gw1_sb, 128, gb1_sb, xt2_sb, False),
                    (xt2_sb, 128, gw2_sb, 128, gb2_sb, xt3_sb, False),
                    (xt3_sb, 128, gw3_sb, 256, gb3_sb, xt4_sb, True),
                ]
                if "g" not in PHASES:
                    layer_cfg = []
                for xin_sb, fin, gw_sb, fout, gb_sb, xout_sb, chunked in layer_cfg:
                    for g in range(G):
                        gcols = slice(g * MAXN, (g + 1) * MAXN)
                        pxw = psA.tile([MAXN, fout], f32, tag="xw")
                        nc.tensor.matmul(
                            pxw,
                            mmt(xin_sb[:fin, gcols]),
                            mmt(gw_sb[:fin, :]),
                            start=True,
                            stop=True,
                        )
                        xw_sb = work.tile([MAXN, fout], f32, tag="xwsb")
                        nc.vector.tensor_copy(xw_sb, pxw)
                        for fo in range(fout // 128):
                            ph = psA.tile([128, MAXN], f32, tag="hT")
                            nc.tensor.matmul(
                                ph,
                                mmt(xw_sb[:, fo * 128 : (fo + 1) * 128]),
                                mmt(mt_sb[:, gcols]),
                                start=True,
                                stop=True,
                            )
                            if chunked:
                                dst = xout_sb[:, fo, gcols]
                                bias = gb_sb[:, fo : fo + 1]
                            else:
                                dst = xout_sb[:, gcols]
                                bias = gb_sb[:, 0:1]
                            nc.scalar.activation(
                                out=dst, in_=ph, func=ACT.Relu, bias=bias
                            )

                # zero the pad node slots (cols 40..44 of each graph) before fc1
                pad_ap = None if "g" not in PHASES else xt4_sb.rearrange(
                    "p c (g s) -> p c g s", g=G
                )[:, :, :, NPG:MAXN]
                if pad_ap is not None:
                    nc.vector.memset(pad_ap, 0.0)

                # ============ fc1 / fc2 over dense nodes (feature-major) ============
                HN = NODES // 2  # 360
                for half in range(2 if "g" in PHASES else 0):
                    nsl = slice(half * HN, (half + 1) * HN)
                    for mc in range(8):
                        pf = psA.tile([128, HN], f32, tag="pt1")
                        for kc in range(2):
                            nc.tensor.matmul(
                                pf,
                                mmt(fc1w_sb[:, kc, mc * 128 : (mc + 1) * 128]),
                                mmt(xt4_sb[:, kc, nsl]),
                                start=(kc == 0),
                                stop=(kc == 1),
                            )
                        nc.scalar.activation(
                            out=y1_sb[:, mc, nsl],
                            in_=pf,
                            func=ACT.Relu,
                            bias=fc1b_sb[:, mc : mc + 1],
                        )
                for half in range(2 if "g" in PHASES else 0):
                    nsl = slice(half * HN, (half + 1) * HN)
                    pf = psA.tile([128, HN], f32, tag="pt2")
                    for kc in range(8):
                        nc.tensor.matmul(
                            pf,
                            mmt(fc2w_sb[:, kc, :]),
                            mmt(y1_sb[:, kc, nsl]),
                            start=(kc == 0),
                            stop=(kc == 7),
                        )
                    nc.scalar.activation(
                        out=xc_sb[:, nsl],
                        in_=pf,
                        func=ACT.Relu,
                        bias=fc2b_sb[:, 0:1],
                    )

            # ============ co-attention, per graph ============
            with tc.tile_pool(name="psumC", bufs=2, space="PSUM") as psC, \
                 tc.tile_pool(name="psumC1", bufs=1, space="PSUM") as psC1, \
                 tc.tile_pool(name="psumS", bufs=3, space="PSUM") as psS:
                for g in range(G if "c" in PHASES else 0):
                    gcols = slice(g * MAXN, (g + 1) * MAXN)
                    lcols = slice(g * LP, (g + 1) * LP)
                    xc_g = xc_sb[:, gcols]          # [128, 45]
                    tt_g = ttall_sb[:, lcols]       # [128, 512]

                    # U = (t @ W_b).T = W_b.T @ t.T  -> [128, 512]
                    pu = psC.tile([128, LP], f32, tag="u")
                    nc.tensor.matmul(pu, mmt(wb_sb), mmt(tt_g), start=True, stop=True)
                    u_sb = work.tile([128, LP], f32, tag="usb")
                    nc.vector.tensor_copy(u_sb, pu)

                    if CSTEP < 2:
                        continue
                    # C.T (s-major): tanh(xc.T @ U) -> [45, 512]
                    pcst = psC1.tile([MAXN, LP], f32, tag="cst")
                    nc.tensor.matmul(pcst, mmt(xc_g), mmt(u_sb), start=True, stop=True)
                    cst_sb = work.tile([MAXN, LP], f32, tag="cstsb")
                    nc.scalar.activation(out=cst_sb, in_=pcst, func=ACT.Tanh)

                    if CSTEP < 3:
                        continue
                    # C (l-major chunks): tanh(U[:,lc].T @ xc) -> 4x [128, 45]
                    cl_sb = work.tile([128, 4, MAXN], f32, tag="clsb")
                    for lc in range(4):
                        pcl = psS.tile([128, MAXN], f32, tag="small")
                        nc.tensor.matmul(
                            pcl,
                            mmt(u_sb[:, lc * 128 : (lc + 1) * 128]),
                            mmt(xc_g),
                            start=True,
                            stop=True,
                        )
                        nc.scalar.activation(
                            out=cl_sb[:, lc, :], in_=pcl, func=ACT.Tanh
                        )

                    if CSTEP < 4:
                        continue
                    # WptT chunks: (t @ W_p.T) -> 4x [128, 32]
                    wptT_sb = work.tile([128, 4, 32], f32, tag="wptsb")
                    for lc in range(4):
                        pwp = psS.tile([128, 32], f32, tag="small")
                        nc.tensor.matmul(
                            pwp,
                            mmt(tt_g[:, lc * 128 : (lc + 1) * 128]),
                            mmt(wpt_sb),
                            start=True,
                            stop=True,
                        )
                        nc.vector.tensor_copy(wptT_sb[:, lc, :], pwp)

                    # WcxT = xc.T @ W_c.T -> [45, 32]
                    pwcx = psS.tile([MAXN, 32], f32, tag="small")
                    nc.tensor.matmul(pwcx, mmt(xc_g), mmt(wct_sb), start=True, stop=True)
                    wcx_sb = work.tile([MAXN, 32], f32, tag="wcxsb")
                    nc.vector.tensor_copy(wcx_sb, pwcx)

                    if CSTEP < 5:
                        continue
                    # H_p = tanh(Wpt + Wcx @ C.T) -> [32, 512]
                    php = psC1.tile([32, LP], f32, tag="hp")
                    nc.tensor.matmul(php, mmt(wpt_sb), mmt(tt_g), start=True, stop=False)
                    nc.tensor.matmul(php, mmt(wcx_sb), mmt(cst_sb), start=False, stop=True)
                    hp_sb = work.tile([32, LP], f32, tag="hpsb")
                    nc.scalar.activation(out=hp_sb, in_=php, func=ACT.Tanh)

                    if CSTEP < 6:
                        continue
                    # H_c = tanh(Wcx + Wpt @ C) -> [32, 45]
                    phc = psS.tile([32, MAXN], f32, tag="small")
                    nc.tensor.matmul(phc, mmt(wct_sb), mmt(xc_g), start=True, stop=False)
                    for lc in range(4):
                        nc.tensor.matmul(
                            phc,
                            mmt(wptT_sb[:, lc, :]),
                            mmt(cl_sb[:, lc, :]),
                            start=False,
                            stop=(lc == 3),
                        )
                    hc_sb = work.tile([32, MAXN], f32, tag="hcsb")
                    nc.scalar.activation(out=hc_sb, in_=phc, func=ACT.Tanh)

                    if CSTEP < 7:
                        continue
                    # attention scores + softmax + weighted sums
                    def softmax_row(score_ps, width, tag):
                        mx = work.tile([1, 1], f32, tag=tag + "mx")
                        nc.vector.reduce_max(mx, score_ps, axis=mybir.AxisListType.X)
                        e = work.tile([1, width], f32, tag=tag + "e")
                        nc.vector.tensor_scalar_sub(e, score_ps, mx)
                        nc.scalar.activation(out=e, in_=e, func=ACT.Exp)
                        sm = work.tile([1, 1], f32, tag=tag + "sm")
                        nc.vector.reduce_sum(sm, e, axis=mybir.AxisListType.X)
                        rv = work.tile([1, 1], f32, tag=tag + "rv")
                        nc.vector.reciprocal(rv, sm)
                        a = work.tile([1, width], f32, tag=tag + "a")
                        nc.vector.tensor_scalar_mul(a, e, rv)
                        return a

                    psc = psS.tile([1, MAXN], f32, tag="small")
                    nc.tensor.matmul(psc, mmt(whc_sb), mmt(hc_sb), start=True, stop=True)
                    a_c = softmax_row(psc, MAXN, "sc")

                    psp = psC1.tile([1, LP], f32, tag="hp")
                    nc.tensor.matmul(psp, mmt(whp_sb), mmt(hp_sb), start=True, stop=True)
                    a_p = softmax_row(psp, LP, "sp")

                    if CSTEP < 8:
                        continue
                    # broadcast attention across partitions via PE (ones[1,128])
                    scr = work.tile([128, LP], f32, tag="scr")
                    pbc = psS.tile([128, MAXN], f32, tag="small")
                    nc.tensor.matmul(pbc, mmt(ones_sb), mmt(a_c), start=True, stop=True)
                    if CSTEP < 9:
                        nc.vector.tensor_copy(scr[:, :MAXN], pbc)
                        continue
                    nc.vector.tensor_mul(scr[:, :MAXN], xc_g, pbc)
                    nc.vector.reduce_sum(
                        cpc_sb[:, g : g + 1], scr[:, :MAXN], axis=mybir.AxisListType.X
                    )
                    pbp = psC.tile([128, LP], f32, tag="u")
                    nc.tensor.matmul(pbp, mmt(ones_sb), mmt(a_p), start=True, stop=True)
                    nc.vector.tensor_mul(scr, tt_g, pbp)
                    nc.vector.reduce_sum(
                        cpp_sb[:, g : g + 1], scr, axis=mybir.AxisListType.X
                    )

                # ============ head: cat1 -> cat2 -> out ============
                for mc in range(8 if "h" in PHASES else 0):
                    pf = psS.tile([128, G], f32, tag="small")
                    nc.tensor.matmul(
                        pf, mmt(c1w_sb[:, 0, mc * 128 : (mc + 1) * 128]), mmt(cpc_sb),
                        start=True, stop=False,
                    )
                    nc.tensor.matmul(
                        pf, mmt(c1w_sb[:, 1, mc * 128 : (mc + 1) * 128]), mmt(cpp_sb),
                        start=False, stop=True,
                    )
                    nc.scalar.activation(
                        out=y1h_sb[:, mc, :], in_=pf, func=ACT.Relu,
                        bias=c1b_sb[:, mc : mc + 1],
                    )
                for mc in range(4 if "h" in PHASES else 0):
                    pf = psS.tile([128, G], f32, tag="small")
                    for kc in range(8):
                        nc.tensor.matmul(
                            pf, mmt(c2w_sb[:, kc, mc * 128 : (mc + 1) * 128]),
                            mmt(y1h_sb[:, kc, :]),
                            start=(kc == 0), stop=(kc == 7),
                        )
                    nc.scalar.activation(
                        out=y2h_sb[:, mc, :], in_=pf, func=ACT.Relu,
                        bias=c2b_sb[:, mc : mc + 1],
                    )
                pf = psS.tile([1, G], f32, tag="small")
                for kc in range(4 if "h" in PHASES else 0):
                    nc.tensor.matmul(
                        pf, mmt(ow_sb[:, kc : kc + 1]), mmt(y2h_sb[:, kc, :]),
                        start=(kc == 0), stop=(kc == 3),
                    )
                if "h" in PHASES:
                    nc.scalar.activation(
                        out=outh_sb, in_=pf, func=ACT.Identity, bias=ob_sb[0:1, 0:1]
                    )
                else:
                    nc.vector.memset(outh_sb, 0.0)
                nc.sync.dma_start(out=out_d[:, :], in_=outh_sb)

    nc.compile()
    return nc


def _preprocess(inputs):
    """Host-side sharding + index preprocessing. Returns per-core input maps."""
    import ml_dtypes

    x = np.asarray(inputs["x"], np.float32)
    edge_index = np.asarray(inputs["edge_index"]).astype(np.int64)
    target_id = np.asarray(inputs["target_id"]).astype(np.int64)
    batch = np.asarray(inputs["batch"]).astype(np.int64)
    proteins = np.asarray(inputs["proteins"], np.float32)

    n = x.shape[0]
    counts = np.bincount(batch, minlength=B)
    assert counts.max() <= MAXN, "graph larger than MAXN"
    starts = np.concatenate([[0], np.cumsum(counts)[:-1]])
    pos = np.arange(n) - starts[batch]

    src, dst = edge_index[0], edge_index[1]
    assert np.all(batch[src] == batch[dst]), "cross-graph edge"
    deg = np.bincount(dst, minlength=n).astype(np.float32) + 1.0
    dinv = 1.0 / np.sqrt(deg)

    # M[g, d, s] = dinv[d]*dinv[s]*#edges(s->d) + diag(1/deg)
    M = np.zeros((B, MAXN, MAXN), np.float32)
    np.add.at(M, (batch[dst], pos[dst], pos[src]),
              (dinv[src] * dinv[dst]).astype(np.float32))
    M[batch, pos, pos] += (1.0 / deg).astype(np.float32)
    MT = np.ascontiguousarray(np.swapaxes(M, 1, 2))  # [B, s, d]

    Xd = np.zeros((B, MAXN, x.shape[1]), np.float32)
    Xd[batch, pos] = x

    f32 = np.float32
    tt_np = ml_dtypes.bfloat16 if TT_BF16 else f32

    shared = dict(
        gw1=np.ascontiguousarray(inputs["gW1"], dtype=f32),
        gw2=np.ascontiguousarray(inputs["gW2"], dtype=f32),
        gw3=np.ascontiguousarray(inputs["gW3"], dtype=f32),
        gb1=np.asarray(inputs["gb1"], f32).reshape(128, 1),
        gb2=np.asarray(inputs["gb2"], f32).reshape(128, 1),
        gb3=np.ascontiguousarray(np.asarray(inputs["gb3"], f32).reshape(2, 128).T),
        fc1w=np.ascontiguousarray(
            np.asarray(inputs["fc1_W"], f32).reshape(2, 128, 1024).transpose(1, 0, 2)),
        fc1b=np.ascontiguousarray(np.asarray(inputs["fc1_b"], f32).reshape(8, 128).T),
        fc2w=np.ascontiguousarray(
            np.asarray(inputs["fc2_W"], f32).reshape(8, 128, 128).transpose(1, 0, 2)),
        fc2b=np.asarray(inputs["fc2_b"], f32).reshape(128, 1),
        b1w=np.ascontiguousarray(
            np.asarray(inputs["bert1_W"], f32).reshape(10, 128, 128)
            .transpose(1, 0, 2)).astype(tt_np),
        b1b=np.asarray(inputs["bert1_b"], f32).reshape(128, 1),
        b2w=np.ascontiguousarray(inputs["bert2_W"], dtype=f32),
        b2b=np.asarray(inputs["bert2_b"], f32).reshape(128, 1),
        wb=np.ascontiguousarray(inputs["W_b"], dtype=f32),
        wct=np.ascontiguousarray(np.asarray(inputs["W_c"], f32).T),
        wpt=np.ascontiguousarray(np.asarray(inputs["W_p"], f32).T),
        whc=np.ascontiguousarray(inputs["w_hc"], dtype=f32),
        whp=np.ascontiguousarray(inputs["w_hp"], dtype=f32),
        c1w=np.ascontiguousarray(
            np.asarray(inputs["cat1_W"], f32).reshape(2, 128, 1024).transpose(1, 0, 2)),
        c1b=np.ascontiguousarray(np.asarray(inputs["cat1_b"], f32).reshape(8, 128).T),
        c2w=np.ascontiguousarray(
            np.asarray(inputs["cat2_W"], f32).reshape(8, 128, 512).transpose(1, 0, 2)),
        c2b=np.ascontiguousarray(np.asarray(inputs["cat2_b"], f32).reshape(4, 128).T),
        ow=np.ascontiguousarray(np.asarray(inputs["out_W"], f32).reshape(4, 128).T),
        ob=np.asarray(inputs["out_b"], f32).reshape(1, 1),
    )

    in_maps = []
    for c in range(N_CORES):
        gs = slice(c * G, (c + 1) * G)
        xt_c = np.ascontiguousarray(
            Xd[gs].transpose(2, 0, 1).reshape(x.shape[1], NODES))
        mt_c = np.ascontiguousarray(
            MT[gs].transpose(1, 0, 2).reshape(MAXN, NODES))
        # t.T per protein, laid out [G, 128, 10, 512]: [p, kc, l] = t[l, kc*128+p]
        t_c = proteins[target_id[gs]]                      # [G, 512, 1280]
        tt_c = np.ascontiguousarray(
            t_c.transpose(0, 2, 1)                          # [G, 1280, 512]
            .reshape(G, KC_BERT, 128, LP)
            .transpose(0, 2, 1, 3)).astype(tt_np)           # [G, 128, 10, 512]
        m = dict(shared)
        m.update(xt=xt_c, mt=mt_c, tt=tt_c)
        in_maps.append(m)
    return in_maps


LAST_EXEC_NS = None


def kernel(**inputs) -> np.ndarray:
    global LAST_EXEC_NS
    from concourse.bass_utils import run_bass_kernel_spmd

    key = (USE_F32R, TT_BF16, PHASES, CSTEP)
    if key not in _PROGRAM_CACHE:
        _PROGRAM_CACHE[key] = _build_program()
    nc = _PROGRAM_CACHE[key]

    in_maps = _preprocess(inputs)
    trace = os.environ.get("GCNBERT_TRACE", "0") == "1"
    res = run_bass_kernel_spmd(nc, in_maps, list(range(N_CORES)), trace=trace)
    LAST_EXEC_NS = res.exec_time_ns

    out = np.empty((B, 1), np.float32)
    for c in range(N_CORES):
        out[c * G : (c + 1) * G, 0] = res.results[c]["out"][0]
    return out
